# revision 10
# baseline (speedup 1.0000x reference)
"""Bass/Trainium2 kernel for nn_DabDeformableDetrTransformer (nms_detection).

Strategy (data-parallel over batch, 1 image per NeuronCore):
  Device phase A (approx, bf16): scores for all S=19947 rows without
    materializing LayerNorm, via host-precomposed weights:
      cls_j = rs * (mem @ WQ)_j (+affine terms), rs from fused stats.
  Device phase L1: top-8 per partition -> 1024 candidate rows.
  Device phase B: exact fp32 rescore (enc matmul + LN + cls) of candidates.
  Device phase C: exact ordered top-24 via max8/match_replace on [1,1024].
  Device phase B2: fp32 coords MLP on the 24 winners only (gather commutes
    with row-wise MLP).
  Host: shard/unshard, weight precompose, proposal grid (index arithmetic
    from mask), sigmoid/trunc/NMS on the final 20 boxes (exact IEEE numpy,
    ~micro work).
"""

import sys

for _p in ("/opt/trn_rl_repo", "/opt/pypackages"):
    if _p not in sys.path:
        sys.path.insert(0, _p)

import numpy as np
import ml_dtypes

import concourse.bacc as bacc
import concourse.bass as bass
import concourse.mybir as mybir
from concourse.tile import TileContext
from concourse.bass_utils import run_bass_kernel_spmd
from concourse.masks import make_identity

F32 = mybir.dt.float32
BF16 = mybir.dt.bfloat16
I32 = mybir.dt.int32
I16 = mybir.dt.int16
U32 = mybir.dt.uint32
AF = mybir.ActivationFunctionType
OP = mybir.AluOpType

SPATIAL_SHAPES = ((100, 150), (50, 75), (25, 38), (13, 19))
S = 19947
NT = 156            # S tiles of 128 (156*128 = 19968)
SP = NT * 128
C = 256
NCLS = 91
TOPK = 20
NSEL = 24           # device returns top-24 (3 max8 rounds); host uses 20
NCAND = 1024        # candidate rows after L1 (top-8 per partition)
NB = NCAND // 128   # 8 candidate tiles
EPS = 1e-5
NEG = -1e30


# ----------------------------------------------------------------------------
# Host-side reference-port helpers (numpy float32, matching jax semantics)
# ----------------------------------------------------------------------------

def _host_proposals(mask_row):
    """Port of _gen_proposals grid math for ONE image.

    Returns op_final [S,4] f32 (+inf on invalid) and vmask [S] f32 (1.0 where
    memory row is kept).
    """
    m = mask_row.astype(bool)
    ops = []
    cur = 0
    for lvl, (H, W) in enumerate(SPATIAL_SHAPES):
        mm = m[cur:cur + H * W].reshape(H, W)
        valid_H = np.float32((~mm[:, 0]).sum())
        valid_W = np.float32((~mm[0, :]).sum())
        gy, gx = np.meshgrid(np.arange(H, dtype=np.float32),
                             np.arange(W, dtype=np.float32), indexing="ij")
        grid = np.stack([gx, gy], -1)
        scale = np.array([valid_W, valid_H], np.float32).reshape(1, 1, 2)
        grid = (grid + np.float32(0.5)) / scale
        wh = np.full_like(grid, np.float32(0.05 * 2.0 ** lvl))
        ops.append(np.concatenate([grid, wh], -1).reshape(-1, 4))
        cur += H * W
    op = np.concatenate(ops, 0).astype(np.float32)
    valid = np.all((op > 0.01) & (op < 0.99), -1)
    op_safe = np.where(valid[:, None], op, np.float32(0.5))
    op_logit = np.log(op_safe / (np.float32(1.0) - op_safe)).astype(np.float32)
    invalid = m | ~valid
    op_final = np.where(invalid[:, None], np.float32(np.inf), op_logit)
    vmask = (~invalid).astype(np.float32)
    return op_final.astype(np.float32), vmask


def _host_nms_keep(boxes, scores, thr):
    """Port of _nms_keep (scores already descending -> order = arange)."""
    order = np.argsort(-scores, kind="stable")
    b = boxes[order]
    x1, y1, x2, y2 = b[:, 0], b[:, 1], b[:, 2], b[:, 3]
    areas = (x2 - x1) * (y2 - y1)
    xx1 = np.maximum(x1[:, None], x1[None, :])
    yy1 = np.maximum(y1[:, None], y1[None, :])
    xx2 = np.minimum(x2[:, None], x2[None, :])
    yy2 = np.minimum(y2[:, None], y2[None, :])
    inter = np.maximum(xx2 - xx1, 0.0) * np.maximum(yy2 - yy1, 0.0)
    with np.errstate(divide="ignore", invalid="ignore"):
        iou = inter / (areas[:, None] + areas[None, :] - inter)
    n = boxes.shape[0]
    idx = np.arange(n)
    supp = np.zeros(n, bool)
    for i in range(n):
        alive = ~supp[i]
        with np.errstate(invalid="ignore"):
            supp = supp | (alive & (iou[i] > thr) & (idx > i))
    keep = np.zeros(n, bool)
    keep[order] = ~supp
    return keep


def _sigmoid_np(x):
    x = x.astype(np.float32)
    pos = x >= 0
    out = np.empty_like(x)
    out[pos] = np.float32(1.0) / (np.float32(1.0) + np.exp(-x[pos]))
    ex = np.exp(x[~pos])
    out[~pos] = ex / (np.float32(1.0) + ex)
    return out


# ----------------------------------------------------------------------------
# Device program
# ----------------------------------------------------------------------------

def build_program(flags):
    """Build the per-core Bass program. flags: dict of bools
    {benc, ln_affine, bcls, b1, b2, b3, affine_d, affine_c}."""
    nc = bacc.Bacc("TRN2", target_bir_lowering=False, debug=False)

    def din(name, shape, dt=F32):
        return nc.dram_tensor(name, shape, dt, kind="ExternalInput")

    # --- external inputs -----------------------------------------------------
    memT = din("memT", [C, SP], BF16)          # transposed image, bf16
    mem = din("mem", [S, C])                   # natural layout (gather source)
    opv = din("opv", [S, 64])                  # cols 0:4 op_final, col 4 vmask
    maskpen = din("maskpen", [128, NT])        # -1e30 at masked/pad rows
    wencA = din("wencA", [2, 128, C], BF16)    # W_enc bf16 (approx)
    waug = din("waug", [2, 128, 93], BF16)     # [WQ | wbar | W_enc@b_enc] bf16
    wencX = din("wencX", [2, 128, C])          # W_enc fp32 (exact)
    wcls = din("wcls", [2, 128, NCLS])
    w1 = din("w1", [2, 128, C])
    w2 = din("w2", [2, 128, C])
    w3 = din("w3", [2, 128, 4])
    # broadcast consts (always bound; some unused in fast path)
    benc_bc = din("benc_bc", [128, C])
    g_bc = din("g_bc", [128, C])
    lnb_bc = din("lnb_bc", [128, C])
    bcls_bc = din("bcls_bc", [128, NCLS])
    b1_bc = din("b1_bc", [128, C])
    b2_bc = din("b2_bc", [128, C])
    b3_bc = din("b3_bc", [128, 4])
    d_bc = din("d_bc", [128, NCLS])            # approx affine d (general path)
    c_bc = din("c_bc", [128, NCLS])            # approx affine c (general path)

    # --- outputs / internal DRAM --------------------------------------------
    out_fin = nc.dram_tensor("out_fin", [NSEL, 96], F32, kind="ExternalOutput")
    cand_dram = nc.dram_tensor("cand_dram", [NCAND], F32)
    final_dram = nc.dram_tensor("final_dram", [NCAND, 128], F32)
    outmem_dram = nc.dram_tensor("outmem_dram", [NCAND, C], F32)
    s1024_dram = nc.dram_tensor("s1024_dram", [NCAND], F32)
    fidx_dram = nc.dram_tensor("fidx_dram", [128], F32)

    slow_c = flags["affine_c"]  # per-tile full assembly needed

    from contextlib import ExitStack
    with TileContext(nc) as tc, ExitStack() as stk:
        p1 = stk.enter_context(tc.tile_pool(name="persist", bufs=1))
        pw = stk.enter_context(tc.tile_pool(name="weights", bufs=1))
        pmt = stk.enter_context(tc.tile_pool(name="memtiles", bufs=3))
        pscr = stk.enter_context(tc.tile_pool(name="scratch", bufs=2))
        ppz = stk.enter_context(tc.tile_pool(name="psZ", bufs=2, space="PSUM"))
        ppq = stk.enter_context(tc.tile_pool(name="psQ", bufs=2, space="PSUM"))
        ppt = stk.enter_context(tc.tile_pool(name="psT", bufs=2, space="PSUM"))

        # --- constants ------------------------------------------------------
        ident = p1.tile([128, 128], F32)
        make_identity(nc, ident)
        pidx_i = p1.tile([128, 1], I32)
        nc.gpsimd.iota(pidx_i, pattern=[[0, 1]], base=0, channel_multiplier=1)
        pidxf = p1.tile([128, 1], F32)
        nc.vector.tensor_copy(pidxf, pidx_i)
        eps_t = p1.tile([128, 1], F32)
        nc.vector.memset(eps_t, EPS)

        def load(pool, dram_ap, shape, dt=F32, tag=None):
            t = pool.tile(shape, dt, tag=tag)
            nc.sync.dma_start(out=t[:, :] if len(shape) == 2 else t, in_=dram_ap)
            return t

        wencA_sb = [load(pw, wencA[k], [128, C], BF16, tag=f"wencA{k}") for k in range(2)]
        waug_sb = [load(pw, waug[k], [128, 93], BF16, tag=f"waug{k}") for k in range(2)]
        wencX_sb = [load(pw, wencX[k], [128, C], tag=f"wencX{k}") for k in range(2)]
        wcls_sb = [load(pw, wcls[k], [128, NCLS], tag=f"wcls{k}") for k in range(2)]
        w1_sb = [load(pw, w1[k], [128, C], tag=f"w1_{k}") for k in range(2)]
        w2_sb = [load(pw, w2[k], [128, C], tag=f"w2_{k}") for k in range(2)]
        w3_sb = [load(pw, w3[k], [128, 4], tag=f"w3_{k}") for k in range(2)]
        maskpen_sb = load(pw, maskpen.ap(), [128, NT], tag="maskpen")
        benc_sb = load(pw, benc_bc.ap(), [128, C], tag="benc") if flags["benc"] else None
        g_sb = load(pw, g_bc.ap(), [128, C], tag="g") if flags["ln_affine"] else None
        lnb_sb = load(pw, lnb_bc.ap(), [128, C], tag="lnb") if flags["ln_affine"] else None
        bcls_sb = load(pw, bcls_bc.ap(), [128, NCLS], tag="bcls") if flags["bcls"] else None
        b1_sb = load(pw, b1_bc.ap(), [128, C], tag="b1") if flags["b1"] else None
        b2_sb = load(pw, b2_bc.ap(), [128, C], tag="b2") if flags["b2"] else None
        b3_sb = load(pw, b3_bc.ap(), [128, 4], tag="b3") if flags["b3"] else None
        d_sb = load(pw, d_bc.ap(), [128, NCLS], tag="d") if flags["affine_d"] else None
        c_sb = load(pw, c_bc.ap(), [128, NCLS], tag="c") if slow_c else None

        # --- persistent stat rows -------------------------------------------
        sumsq_sb = p1.tile([128, NT], F32)
        mcol_sb = p1.tile([128, NT], F32)
        maxq_sb = p1.tile([128, NT], F32)
        scores_sb = p1.tile([128, NT], F32)

        # ==================== Phase A: approx scores ========================
        MAC = 4  # S-tiles per DMA macro tile
        for j in range((NT + MAC - 1) // MAC):
            t0 = j * MAC
            ntl = min(MAC, NT - t0)
            w = ntl * 128
            mtA = pmt.tile([128, MAC * 128], BF16, tag="mtA")
            mtB = pmt.tile([128, MAC * 128], BF16, tag="mtB")
            nc.sync.dma_start(out=mtA[:, 0:w], in_=memT[0:128, t0 * 128:t0 * 128 + w])
            nc.sync.dma_start(out=mtB[:, 0:w], in_=memT[128:256, t0 * 128:t0 * 128 + w])
            for t2 in range(ntl):
                t = t0 + t2
                sl = slice(t2 * 128, (t2 + 1) * 128)
                psZ = ppz.tile([128, C], F32, tag="psZ")
                nc.tensor.matmul(psZ[:, :], mtA[:, sl], wencA_sb[0][:, :], start=True, stop=False)
                nc.tensor.matmul(psZ[:, :], mtB[:, sl], wencA_sb[1][:, :], start=False, stop=True)
                psQ = ppq.tile([128, 93], F32, tag="psQ")
                nc.tensor.matmul(psQ[:, :], mtA[:, sl], waug_sb[0][:, :], start=True, stop=False)
                nc.tensor.matmul(psQ[:, :], mtB[:, sl], waug_sb[1][:, :], start=False, stop=True)
                # row sum of Z^2 -> sumsq col t (ScalarE), scrap full output
                sq = pscr.tile([128, C], F32, tag="sq")
                nc.scalar.activation(out=sq[:, :], in_=psZ[:, :], func=AF.Square,
                                     accum_out=sumsq_sb[:, t:t + 1])
                nc.vector.tensor_copy(out=mcol_sb[:, t:t + 1], in_=psQ[:, 91:92])
                if not slow_c:
                    if flags["affine_d"]:
                        qd = pscr.tile([128, NCLS], F32, tag="qd")
                        nc.vector.tensor_tensor(out=qd[:, :], in0=psQ[:, 0:NCLS],
                                                in1=d_sb[:, :], op=OP.add)
                        nc.vector.reduce_max(out=maxq_sb[:, t:t + 1], in_=qd[:, :],
                                             axis=mybir.AxisListType.X)
                    else:
                        nc.vector.reduce_max(out=maxq_sb[:, t:t + 1], in_=psQ[:, 0:NCLS],
                                             axis=mybir.AxisListType.X)
                else:
                    # general path: need rs per tile before max; done below in
                    # a second pass over stored Q. Store Q to scratch DRAM not
                    # implemented -- instead compute per-tile with stats ops.
                    # (slow, correctness-only path)
                    m2t = pscr.tile([128, 1], F32, tag="m2t")
                    nc.vector.scalar_tensor_tensor(out=m2t, in0=mcol_sb[:, t:t + 1],
                                                   scalar=1.0, in1=mcol_sb[:, t:t + 1],
                                                   op0=OP.bypass, op1=OP.mult)
                    vet = pscr.tile([128, 1], F32, tag="vet")
                    nc.vector.scalar_tensor_tensor(out=vet, in0=sumsq_sb[:, t:t + 1],
                                                   scalar=1.0 / C, in1=m2t,
                                                   op0=OP.mult, op1=OP.subtract)
                    sdt = pscr.tile([128, 1], F32, tag="sdt")
                    nc.scalar.activation(out=sdt, in_=vet, func=AF.Sqrt, bias=eps_t[:, 0:1])
                    rst = pscr.tile([128, 1], F32, tag="rst")
                    nc.vector.reciprocal(rst, sdt)
                    qq = pscr.tile([128, NCLS], F32, tag="qq")
                    if flags["affine_d"]:
                        nc.vector.tensor_tensor(out=qq, in0=psQ[:, 0:NCLS], in1=d_sb[:, :], op=OP.add)
                        nc.vector.tensor_scalar(out=qq, in0=qq, scalar1=rst[:, 0:1],
                                                scalar2=None, op0=OP.mult)
                    else:
                        nc.vector.tensor_scalar(out=qq, in0=psQ[:, 0:NCLS], scalar1=rst[:, 0:1],
                                                scalar2=None, op0=OP.mult)
                    nc.vector.tensor_tensor(out=qq, in0=qq, in1=c_sb[:, :], op=OP.add)
                    nc.vector.reduce_max(out=scores_sb[:, t:t + 1], in_=qq,
                                         axis=mybir.AxisListType.X)

        # --- tail: vectorized rs + score assembly ---------------------------
        if not slow_c:
            m2 = pscr.tile([128, NT], F32, tag="m2")
            nc.vector.scalar_tensor_tensor(out=m2, in0=mcol_sb, scalar=1.0,
                                           in1=mcol_sb, op0=OP.bypass, op1=OP.mult)
            ve = pscr.tile([128, NT], F32, tag="ve")
            nc.vector.scalar_tensor_tensor(out=ve, in0=sumsq_sb, scalar=1.0 / C,
                                           in1=m2, op0=OP.mult, op1=OP.subtract)
            sd = pscr.tile([128, NT], F32, tag="sd")
            nc.scalar.activation(out=sd, in_=ve, func=AF.Sqrt, bias=eps_t[:, 0:1])
            rs = pscr.tile([128, NT], F32, tag="rs")
            nc.vector.reciprocal(rs, sd)
            tmpS = pscr.tile([128, NT], F32, tag="tmpS")
            nc.vector.tensor_tensor(out=tmpS, in0=maxq_sb, in1=rs, op=OP.mult)
            nc.vector.tensor_tensor(out=scores_sb, in0=tmpS, in1=maskpen_sb, op=OP.add)
        else:
            nc.vector.tensor_tensor(out=scores_sb, in0=scores_sb, in1=maskpen_sb, op=OP.add)

        # ==================== Phase L1: candidates ==========================
        V1 = p1.tile([128, 8], F32)
        I1 = p1.tile([128, 8], U32)
        nc.vector.max(out=V1, in_=scores_sb)
        nc.vector.max_index(out=I1, in_max=V1, in_values=scores_sb)
        I1f = p1.tile([128, 8], F32)
        nc.vector.tensor_copy(I1f, I1)
        GIDX = p1.tile([128, 8], F32)
        nc.vector.tensor_scalar(out=GIDX, in0=I1f, scalar1=128.0,
                                scalar2=pidxf[:, 0:1], op0=OP.mult, op1=OP.add)
        # -> DRAM (flat order i = p*8 + j), read back 16-wrapped, cast int16
        nc.sync.dma_start(out=cand_dram.ap(), in_=GIDX[:, :])
        wrapf = p1.tile([128, 64], F32)
        for g in range(8):
            nc.sync.dma_start(out=wrapf[g * 16:(g + 1) * 16, :],
                              in_=bass.AP(cand_dram, 0, [[1, 16], [16, 64]]))
        wrap_i = p1.tile([128, 64], I16)
        nc.vector.tensor_copy(wrap_i, wrapf)

        # ==================== Phase B1: exact rescore =======================
        memg = p1.tile([128, NB, C], F32)
        opvg = p1.tile([128, NB, 64], F32)
        nc.gpsimd.dma_gather(out_ap=memg[:, :, :], in_ap=mem.ap(), idxs_ap=wrap_i[:, :],
                             num_idxs=NCAND, num_idxs_reg=NCAND, elem_size=C)
        nc.gpsimd.dma_gather(out_ap=opvg[:, :, :], in_ap=opv.ap(), idxs_ap=wrap_i[:, :],
                             num_idxs=NCAND, num_idxs_reg=NCAND, elem_size=64)
        scores1024 = p1.tile([128, NB], F32)

        for b in range(NB):
            # transpose candidate rows -> lhsT
            mTb = []
            for h in range(2):
                pt = ppt.tile([128, 128], F32, tag="ptT")
                nc.tensor.transpose(pt[:, :], memg[:, b, h * 128:(h + 1) * 128], ident[:, :])
                sb = pscr.tile([128, 128], F32, tag=f"mT{h}")
                if h == 0:
                    nc.vector.tensor_copy(sb, pt[:, :])
                else:
                    nc.scalar.copy(sb, pt[:, :])
                mTb.append(sb)
            psZ = ppz.tile([128, C], F32, tag="psZ")
            nc.tensor.matmul(psZ[:, :], mTb[0][:, :], wencX_sb[0][:, :], start=True, stop=False)
            nc.tensor.matmul(psZ[:, :], mTb[1][:, :], wencX_sb[1][:, :], start=False, stop=True)
            # zb = vmask * Z (+ b_enc), rowsum via ACT accum
            zb = pscr.tile([128, C], F32, tag="zb")
            sums = pscr.tile([128, 1], F32, tag="sums")
            if flags["benc"]:
                nc.scalar.activation(out=zb, in_=psZ[:, :], func=AF.Copy,
                                     scale=opvg[:, b, 4:5])
                nc.vector.tensor_tensor(out=zb, in0=zb, in1=benc_sb[:, :], op=OP.add)
                nc.scalar.activation(out=zb, in_=zb, func=AF.Copy, accum_out=sums)
            else:
                nc.scalar.activation(out=zb, in_=psZ[:, :], func=AF.Copy,
                                     scale=opvg[:, b, 4:5], accum_out=sums)
            negm = pscr.tile([128, 1], F32, tag="negm")
            nc.vector.tensor_scalar(out=negm, in0=sums, scalar1=-1.0 / C,
                                    scalar2=None, op0=OP.mult)
            zc = pscr.tile([128, C], F32, tag="zc")
            nc.vector.tensor_scalar(out=zc, in0=zb, scalar1=negm[:, 0:1],
                                    scalar2=None, op0=OP.add)
            sq2 = pscr.tile([128, C], F32, tag="sq2")
            s2 = pscr.tile([128, 1], F32, tag="s2")
            nc.scalar.activation(out=sq2, in_=zc, func=AF.Square, accum_out=s2)
            sdb = pscr.tile([128, 1], F32, tag="sdb")
            nc.scalar.activation(out=sdb, in_=s2, func=AF.Sqrt, scale=1.0 / C, bias=eps_t[:, 0:1])
            rsb = pscr.tile([128, 1], F32, tag="rsb")
            nc.vector.reciprocal(rsb, sdb)
            om = pscr.tile([128, C], F32, tag="om")
            if flags["ln_affine"]:
                nc.vector.scalar_tensor_tensor(out=om, in0=zc, scalar=rsb[:, 0:1],
                                               in1=g_sb[:, :], op0=OP.mult, op1=OP.mult)
                nc.vector.tensor_tensor(out=om, in0=om, in1=lnb_sb[:, :], op=OP.add)
            else:
                nc.vector.tensor_scalar(out=om, in0=zc, scalar1=rsb[:, 0:1],
                                        scalar2=None, op0=OP.mult)
            # cls = om @ Wcls (+bcls)
            omT = []
            for h in range(2):
                pt = ppt.tile([128, 128], F32, tag="ptT")
                nc.tensor.transpose(pt[:, :], om[:, h * 128:(h + 1) * 128], ident[:, :])
                sb = pscr.tile([128, 128], F32, tag=f"oT{h}")
                if h == 0:
                    nc.vector.tensor_copy(sb, pt[:, :])
                else:
                    nc.scalar.copy(sb, pt[:, :])
                omT.append(sb)
            psC = ppq.tile([128, NCLS], F32, tag="psQ")
            nc.tensor.matmul(psC[:, :], omT[0][:, :], wcls_sb[0][:, :], start=True, stop=False)
            nc.tensor.matmul(psC[:, :], omT[1][:, :], wcls_sb[1][:, :], start=False, stop=True)
            cls_sb = pscr.tile([128, NCLS], F32, tag="cls")
            if flags["bcls"]:
                nc.vector.tensor_tensor(out=cls_sb, in0=psC[:, :], in1=bcls_sb[:, :], op=OP.add)
            else:
                nc.scalar.copy(cls_sb, psC[:, :])
            nc.vector.reduce_max(out=scores1024[:, b:b + 1], in_=cls_sb[:, :],
                                 axis=mybir.AxisListType.X)
            # final_dram rows c = b*128+p: [cls | _ | score | op]
            base = b * 128 * 128
            nc.sync.dma_start(
                out=bass.AP(final_dram, base, [[128, 128], [1, NCLS]]),
                in_=cls_sb[:, :])
            nc.sync.dma_start(
                out=bass.AP(final_dram, base + 96, [[128, 128], [1, 4]]),
                in_=opvg[:, b, 0:4])
            nc.sync.dma_start(
                out=bass.AP(outmem_dram, b * 128 * C, [[C, 128], [1, C]]),
                in_=om[:, :])
        nc.sync.dma_start(
            out=bass.AP(final_dram, 95, [[128, 128], [16384, NB]]),
            in_=scores1024[:, :])
        nc.sync.dma_start(out=s1024_dram.ap(), in_=scores1024[:, :])

        # ==================== Phase C: exact ordered top-24 =================
        sflat = [p1.tile([1, NCAND], F32, tag=f"sf{i}", name=f"sf{i}") for i in range(2)]
        # dram i = p*NB + b ; flat row position c = b*128 + p
        nc.sync.dma_start(out=sflat[0][:, :],
                          in_=bass.AP(s1024_dram, 0, [[1, NB], [NB, 128]]))
        V24 = p1.tile([1, NSEL], F32)
        I24 = p1.tile([1, NSEL], U32)
        cur = 0
        for r in range(NSEL // 8):
            nc.vector.max(out=V24[:, r * 8:(r + 1) * 8], in_=sflat[cur][:, :])
            nc.vector.max_index(out=I24[:, r * 8:(r + 1) * 8],
                                in_max=V24[:, r * 8:(r + 1) * 8], in_values=sflat[cur][:, :])
            if r < NSEL // 8 - 1:
                nc.vector.match_replace(out=sflat[1 - cur][:, :],
                                        in_to_replace=V24[:, r * 8:(r + 1) * 8],
                                        in_values=sflat[cur][:, :], imm_value=NEG)
                cur = 1 - cur
        I24f = p1.tile([1, NSEL], F32)
        nc.vector.tensor_copy(I24f, I24)
        padrow = p1.tile([1, 128], F32)
        nc.vector.memset(padrow, -1.0)
        nc.vector.tensor_copy(padrow[:, 0:NSEL], I24f)
        nc.sync.dma_start(out=fidx_dram.ap(), in_=padrow[:, :])
        fwrap = p1.tile([128, 8], F32)
        for g in range(8):
            nc.sync.dma_start(out=fwrap[g * 16:(g + 1) * 16, :],
                              in_=bass.AP(fidx_dram, 0, [[1, 16], [16, 8]]))
        fwrap_i = p1.tile([128, 8], I16)
        nc.vector.tensor_copy(fwrap_i, fwrap)

        fing = p1.tile([128, 1, 128], F32)
        nc.gpsimd.memset(fing[:, :, :], 0.0)
        nc.gpsimd.dma_gather(out_ap=fing[:, :, :], in_ap=final_dram.ap(),
                             idxs_ap=fwrap_i[:, :], num_idxs=128, num_idxs_reg=NSEL,
                             elem_size=128)
        om24 = p1.tile([128, 1, C], F32)
        nc.gpsimd.memset(om24[:, :, :], 0.0)
        nc.gpsimd.dma_gather(out_ap=om24[:, :, :], in_ap=outmem_dram.ap(),
                             idxs_ap=fwrap_i[:, :], num_idxs=128, num_idxs_reg=NSEL,
                             elem_size=C)

        # ==================== Phase B2: coords MLP on top-24 ================
        def mm256(x_sb, w_pair, tag):
            xT = []
            for h in range(2):
                pt = ppt.tile([128, 128], F32, tag="ptT")
                nc.tensor.transpose(pt[:, :], x_sb[:, h * 128:(h + 1) * 128], ident[:, :])
                sb = pscr.tile([128, 128], F32, tag=f"{tag}{h}")
                nc.vector.tensor_copy(sb, pt[:, :])
                xT.append(sb)
            n = w_pair[0].shape[-1]
            ps = ppz.tile([128, n], F32, tag="psZ")
            nc.tensor.matmul(ps[:, :], xT[0][:, :], w_pair[0][:, :], start=True, stop=False)
            nc.tensor.matmul(ps[:, :], xT[1][:, :], w_pair[1][:, :], start=False, stop=True)
            return ps

        om24v = om24[:, 0, :]
        ps1 = mm256(om24v, w1_sb, "h1")
        h1 = pscr.tile([128, C], F32, tag="h1")
        if flags["b1"]:
            nc.vector.tensor_tensor(out=h1, in0=ps1[:, :], in1=b1_sb[:, :], op=OP.add)
            nc.scalar.activation(out=h1, in_=h1, func=AF.Relu)
        else:
            nc.scalar.activation(out=h1, in_=ps1[:, :], func=AF.Relu)
        ps2 = mm256(h1, w2_sb, "h2")
        h2 = pscr.tile([128, C], F32, tag="h2")
        if flags["b2"]:
            nc.vector.tensor_tensor(out=h2, in0=ps2[:, :], in1=b2_sb[:, :], op=OP.add)
            nc.scalar.activation(out=h2, in_=h2, func=AF.Relu)
        else:
            nc.scalar.activation(out=h2, in_=ps2[:, :], func=AF.Relu)
        ps3 = mm256(h2, w3_sb, "co")
        co = pscr.tile([128, 4], F32, tag="co")
        if flags["b3"]:
            nc.vector.tensor_tensor(out=co, in0=ps3[:, :], in1=b3_sb[:, :], op=OP.add)
            nc.vector.tensor_tensor(out=co, in0=co, in1=fing[:, 0, 96:100], op=OP.add)
        else:
            nc.vector.tensor_tensor(out=co, in0=ps3[:, :], in1=fing[:, 0, 96:100], op=OP.add)

        # ==================== outputs =======================================
        nc.sync.dma_start(out=out_fin[0:NSEL, 0:NCLS], in_=fing[0:NSEL, 0, 0:NCLS])
        nc.sync.dma_start(out=out_fin[0:NSEL, 91:95], in_=co[0:NSEL, :])
        nc.sync.dma_start(out=out_fin[0:NSEL, 95:96], in_=fing[0:NSEL, 0, 95:96])

    nc.finalize()
    return nc


# ----------------------------------------------------------------------------
# Host entry
# ----------------------------------------------------------------------------

_PROG_CACHE = {}


def _prep_weights(weights):
    (W_enc, b_enc, ln_g, ln_b, W_cls, b_cls, W1, b1, W2, b2, W3, b3) = weights
    f64 = np.float64
    Wg = W_enc.astype(f64) @ (np.diag(ln_g.astype(f64)) @ W_cls.astype(f64))
    u = ln_g.astype(f64) @ W_cls.astype(f64)
    wbar = W_enc.astype(f64).mean(axis=1)
    WQ = Wg - np.outer(wbar, u)
    e = b_enc.astype(f64) @ (np.diag(ln_g.astype(f64)) @ W_cls.astype(f64))
    bbar = f64(b_enc.astype(f64).mean())
    d = e - bbar * u
    cc = ln_b.astype(f64) @ W_cls.astype(f64) + b_cls.astype(f64)
    wencb = W_enc.astype(f64) @ b_enc.astype(f64)

    waug = np.zeros((C, 93), f64)
    waug[:, 0:NCLS] = WQ
    waug[:, 91] = wbar
    waug[:, 92] = wencb

    def ksplit(W, dt=np.float32):
        return np.ascontiguousarray(W.reshape(2, 128, -1)).astype(dt)

    bc = lambda v, n: np.ascontiguousarray(
        np.broadcast_to(np.asarray(v, np.float32), (128, n)))

    wins = {
        "wencA": ksplit(W_enc.astype(f64), ml_dtypes.bfloat16),
        "waug": ksplit(waug, ml_dtypes.bfloat16),
        "wencX": ksplit(W_enc),
        "wcls": ksplit(W_cls),
        "w1": ksplit(W1),
        "w2": ksplit(W2),
        "w3": ksplit(W3),
        "benc_bc": bc(b_enc, C),
        "g_bc": bc(ln_g, C),
        "lnb_bc": bc(ln_b, C),
        "bcls_bc": bc(b_cls, NCLS),
        "b1_bc": bc(b1, C),
        "b2_bc": bc(b2, C),
        "b3_bc": bc(b3, 4),
        "d_bc": bc(d.astype(np.float32), NCLS),
        "c_bc": bc(cc.astype(np.float32), NCLS),
    }
    flags = {
        "benc": bool(np.any(b_enc != 0)),
        "ln_affine": bool(np.any(ln_g != 1) or np.any(ln_b != 0)),
        "bcls": bool(np.any(b_cls != 0)),
        "b1": bool(np.any(b1 != 0)),
        "b2": bool(np.any(b2 != 0)),
        "b3": bool(np.any(b3 != 0)),
        "affine_d": bool(np.max(np.abs(d)) > 0),
        "affine_c": bool(np.max(np.abs(cc)) > 0),
    }
    return wins, flags


def _prep_image_inputs(mem_i, mask_i, shared_cache):
    key = mask_i.tobytes()
    if key in shared_cache:
        opv, maskpen = shared_cache[key]
    else:
        op_final, vmask = _host_proposals(mask_i)
        opv = np.zeros((S, 64), np.float32)
        opv[:, 0:4] = op_final
        opv[:, 4] = vmask
        maskpen = np.zeros((128, NT), np.float32)
        sflat = np.full(SP, 0.0, np.float32)
        sflat[:S] = np.where(mask_i.astype(bool), np.float32(NEG), np.float32(0.0))
        sflat[S:] = NEG
        maskpen[:, :] = sflat.reshape(NT, 128).T
        shared_cache[key] = (opv, maskpen)
    memT = np.zeros((C, SP), ml_dtypes.bfloat16)
    memT[:, :S] = mem_i.T
    mem_c = mem_i if mem_i.flags.c_contiguous and mem_i.dtype == np.float32 \
        else np.ascontiguousarray(mem_i, dtype=np.float32)
    return {"memT": memT, "mem": mem_c, "opv": opv, "maskpen": maskpen}


def _prep_core_inputs(mem_i, mask_i, weights):
    wins, flags = _prep_weights(weights)
    ins = dict(wins)
    ins.update(_prep_image_inputs(np.asarray(mem_i, np.float32), mask_i, {}))
    return ins, flags


def kernel(memory, mask, WH, W_enc, b_enc, ln_g, ln_b, W_cls, b_cls,
           W1, b1, W2, b2, W3, b3, _return_results=False, _trace=False):
    memory = np.asarray(memory, np.float32)
    mask = np.asarray(mask)
    WH = np.asarray(WH, np.float32)
    weights = tuple(np.asarray(w, np.float32) for w in
                    (W_enc, b_enc, ln_g, ln_b, W_cls, b_cls, W1, b1, W2, b2, W3, b3))
    bs = memory.shape[0]
    assert bs == 8 and memory.shape[1] == S and memory.shape[2] == C

    wins, flags = _prep_weights(weights)
    mem_bf = memory.astype(ml_dtypes.bfloat16)
    shared_cache = {}
    in_maps = []
    for i in range(bs):
        ins = dict(wins)
        ins.update(_prep_image_inputs(memory[i], mask[i], shared_cache))
        ins["memT"][:, :S] = mem_bf[i].T
        in_maps.append(ins)

    key = tuple(sorted(flags.items()))
    if key not in _PROG_CACHE:
        _PROG_CACHE[key] = build_program(flags)
    nc = _PROG_CACHE[key]

    import time as _time
    _t0 = _time.time()
    res = run_bass_kernel_spmd(nc, in_maps, core_ids=list(range(bs)), trace=_trace)
    kernel.last_dispatch_ns = int((_time.time() - _t0) * 1e9)

    out = np.zeros((bs, TOPK, 95), np.float32)
    ref_pts = np.zeros((bs, TOPK, 4), np.float32)
    keep = np.zeros((bs, TOPK), bool)
    top_scores = np.zeros((bs, TOPK), np.float32)
    for i in range(bs):
        fin = np.asarray(res.results[i]["out_fin"])[:TOPK]
        top_cls = fin[:, 0:NCLS]
        top_coords = fin[:, 91:95]
        scr = fin[:, 95]
        rp = _sigmoid_np(top_coords)
        cx, cy, w_, h_ = rp[:, 0], rp[:, 1], rp[:, 2], rp[:, 3]
        xyxy = np.stack([cx - 0.5 * w_, cy - 0.5 * h_,
                         cx + 0.5 * w_, cy + 0.5 * h_], -1).astype(np.float32)
        boxes = np.trunc(xyxy * WH[i][None, :]).astype(np.float32)
        keep[i] = _host_nms_keep(boxes, scr, np.float32(0.5))
        out[i, :, 0:NCLS] = top_cls
        out[i, :, NCLS:95] = top_coords
        ref_pts[i] = rp
        top_scores[i] = scr
    if _return_results:
        return (out, ref_pts, keep, top_scores), res
    return out, ref_pts, keep, top_scores
